# revision 25
# baseline (speedup 1.0000x reference)
"""Trainium2 Bass kernel for nn_Attention_14542759264705.

Dense transformer attention: QKV proj + interleaved RoPE + GQA causal
attention (32 q heads / 8 kv heads, hd=64) + output proj, fp32 in/out.

Sharding: tensor-parallel over kv-head groups across 8 cores. Core c owns
q heads 4c..4c+3 and kv head c; each core computes a partial output and
the host sums the 8 partials.

v2: bf16 datapath (fp32 PSUM accumulation everywhere):
  - x converted to bf16 on host; xT loaded straight into SBUF via
    DMA-transpose (removes 512 PE transposes + 128 ACT evacuations).
  - weights/Q/K/V/probs/attn/masks/output all bf16; rope DVE ops run in
    bf16 2x mode; mask muls 2x.
  - proj: 1024-token chunks, 6 psum accumulators, fast evac to SBUF
    (3 DVE + 3 ACT copies) then rope reads SBUF.
  - scores: per kt one [128,1024] 2-bank psum tile (two heads), single
    paired exp; half-width diagonal tiles compacted to the middle 512
    cols so one exp covers both heads.
  - softmax denominators: reciprocal via ACT ln->exp(-x) (single
    natural_log_exp table set) instead of 3.3us DVE reciprocals.
  - wo: 2-do-group accumulation, evac alternating DVE/ACT, bf16 output
    summed on host in fp32.
"""
import numpy as np

B, S, D = 2, 2048, 2048
T = B * S
NH, NKV, HD = 32, 8, 64
NCORES = 8

_cache = {}


def _build():
    import concourse.bacc as bacc
    import concourse.mybir as mybir
    import concourse.tile as tile
    from concourse.masks import make_identity

    F32 = mybir.dt.float32
    BF16 = mybir.dt.bfloat16
    AF = mybir.ActivationFunctionType

    nc = bacc.Bacc("TRN2", target_bir_lowering=False, debug=False,
                   num_devices=NCORES)
    # Steer the act-table pass to the set holding BOTH exp and ln, so the
    # per-pass ln->exp reciprocals don't thrash ACT_TABLE_LOADs: hide Exp
    # from every other set (the chosen set genuinely contains both, so the
    # emitted program is unchanged in semantics, just uses one table).
    from concourse.hw_specs import get_activation_tables
    tabs = get_activation_tables(nc.m.arch)
    for name, fns in tabs.items():
        if name != "natural_log_exp_and_others":
            fns.discard(AF.Exp)
    x = nc.dram_tensor("x", [T, D], BF16, kind="ExternalInput").ap()
    wqkvT = nc.dram_tensor("wqkvT", [D, 384], BF16, kind="ExternalInput").ap()
    woT = nc.dram_tensor("woT", [256, D], BF16, kind="ExternalInput").ap()
    c4 = nc.dram_tensor("c4", [128, S], BF16, kind="ExternalInput").ap()
    s4 = nc.dram_tensor("s4", [128, S], BF16, kind="ExternalInput").ap()
    maskP = nc.dram_tensor("maskP", [128, 4 * 512], BF16,
                           kind="ExternalInput").ap()
    o = nc.dram_tensor("o", [T, D], BF16, kind="ExternalOutput").ap()

    with tile.TileContext(nc) as tc:
        with tc.tile_pool(name="resident", bufs=1) as res:
            ident = res.tile([128, 128], BF16)
            make_identity(nc, ident[:])
            c4_sb = res.tile([128, S], BF16)
            s4_sb = res.tile([128, S], BF16)
            maskP_sb = res.tile([128, 4 * 512], BF16)

            QRI_A = res.tile([128, T], BF16)   # [h0r h0i h1r h1i] x tokens
            QRI_B = res.tile([128, T], BF16)   # [h2r h2i h3r h3i]
            KRI2 = res.tile([128, T], BF16)    # [Kr Ki Kr Ki]
            Vt_sb = res.tile([128, 32 * 65], BF16)  # kt-tile k at cols k*65
            Vt3 = Vt_sb.rearrange("p (k c) -> p k c", c=65)
            wqkv_r = res.tile([128, 16 * 384], BF16)
            woT_r = res.tile([128, 2 * D], BF16)
            ones32 = res.tile([128, 32], BF16)
            nc.gpsimd.memset(ones32[:], 1.0)
            nc.vector.tensor_copy(Vt3[:, :, 64], ones32[:])

            for d in range(16):
                nc.sync.dma_start(wqkv_r[:, d * 384:(d + 1) * 384],
                                  wqkvT[d * 128:(d + 1) * 128, :])

            # ---------------- phase 1: xT dma, proj, rope --------------
            with tc.tile_pool(name="xTp", bufs=3) as xTp, \
                 tc.tile_pool(name="evacp", bufs=2) as evp, \
                 tc.tile_pool(name="ropet", bufs=2) as rp, \
                 tc.tile_pool(name="projps", bufs=1, space="PSUM") as projp, \
                 tc.tile_pool(name="vtps", bufs=1, space="PSUM") as vtp_pool:

                for J in range(4):            # 1024-token chunks
                    tb = J * 1024
                    bc = (J % 2) * 1024       # within-batch col base
                    xts = []
                    for d in range(16):
                        xt = xTp.tile([128, 1024], BF16, name=f"xt{d % 3}")
                        nc.sync.dma_start_transpose(
                            xt[:], x[tb:tb + 1024, d * 128:(d + 1) * 128])
                        xts.append(xt)
                    if J == 0:
                        nc.sync.dma_start(c4_sb[:], c4[:])
                        nc.sync.dma_start(s4_sb[:], s4[:])
                        nc.sync.dma_start(maskP_sb[:], maskP[:])
                        for t in range(2):
                            nc.sync.dma_start(
                                woT_r[:, t * D:(t + 1) * D],
                                woT[t * 128:(t + 1) * 128, :])
                    ps = {}
                    for ch, nm in ((0, "QR"), (1, "QI"), (2, "KV")):
                        for h in range(2):
                            ps[(ch, h)] = projp.tile(
                                [128, 512], F32, name=f"{nm}{h}")
                    for d in range(16):
                        for ch in range(3):
                            w = wqkv_r[:, d * 384 + ch * 128:
                                       d * 384 + (ch + 1) * 128]
                            for h in range(2):
                                nc.tensor.matmul(
                                    ps[(ch, h)][:],
                                    w, xts[d][:, h * 512:(h + 1) * 512],
                                    start=(d == 0), stop=(d == 15))
    # fast psum evac (bf16), split DVE/ACT; K and V land on
                    # partition-0-based tiles (V transpose needs base 0)
                    qrE = evp.tile([128, 1024], BF16, name="qrE")
                    qiE = evp.tile([128, 1024], BF16, name="qiE")
                    krE = evp.tile([32, 1024], BF16, name="krE")
                    kiE = evp.tile([32, 1024], BF16, name="kiE")
                    vE = evp.tile([64, 1024], BF16, name="vE")
                    for h in range(2):
                        sl = slice(h * 512, (h + 1) * 512)
                        nc.vector.tensor_copy(qrE[:, sl], ps[(0, h)][:])
                        nc.scalar.copy(qiE[:, sl], ps[(1, h)][:])
                        nc.vector.tensor_copy(krE[:, sl], ps[(2, h)][0:32, :])
                        nc.vector.tensor_copy(kiE[:, sl],
                                              ps[(2, h)][32:64, :])
                        nc.scalar.copy(vE[:, sl], ps[(2, h)][64:128, :])
                    # V transposes
                    for i in range(8):
                        vtp = vtp_pool.tile([128, 64], BF16, name="vtp")
                        nc.tensor.transpose(
                            vtp[:], vE[:, i * 128:(i + 1) * 128],
                            ident[0:64, 0:64])
                        nc.scalar.copy(Vt3[:, J * 8 + i, 0:64], vtp[:])
                    # rope (bf16 DVE 2x mode)
                    cs = c4_sb[:, bc:bc + 1024]
                    sn = s4_sb[:, bc:bc + 1024]
                    t1 = rp.tile([128, 1024], BF16, name="t1")
                    t2 = rp.tile([128, 1024], BF16, name="t2")
                    t3 = rp.tile([128, 1024], BF16, name="t3")
                    t4 = rp.tile([128, 1024], BF16, name="t4")
                    qtr = rp.tile([128, 1024], BF16, name="qtr")
                    qti = rp.tile([128, 1024], BF16, name="qti")
                    nc.vector.tensor_mul(t1[:], qrE[:], cs)
                    nc.vector.tensor_mul(t2[:], qiE[:], sn)
                    nc.vector.tensor_mul(t3[:], qrE[:], sn)
                    nc.vector.tensor_mul(t4[:], qiE[:], cs)
                    nc.vector.tensor_sub(qtr[:], t1[:], t2[:])
                    nc.vector.tensor_add(qti[:], t3[:], t4[:])
                    for hh in range(4):
                        dst = QRI_A if hh < 2 else QRI_B
                        base = (hh % 2) * 64
                        nc.vector.tensor_copy(
                            dst[base:base + 32, tb:tb + 1024],
                            qtr[32 * hh:32 * hh + 32, :])
                        nc.vector.tensor_copy(
                            dst[base + 32:base + 64, tb:tb + 1024],
                            qti[32 * hh:32 * hh + 32, :])
                    # K rope: kE rows = [Kr; Ki]; all u tiles base-0 so
                    # the add/sub inputs share a base partition
                    u1 = rp.tile([32, 1024], BF16, name="u1")
                    u2 = rp.tile([32, 1024], BF16, name="u2")
                    u3 = rp.tile([32, 1024], BF16, name="u3")
                    u4 = rp.tile([32, 1024], BF16, name="u4")
                    bsl = slice(bc, bc + 1024)
                    nc.vector.tensor_mul(u1[:], krE[:], c4_sb[0:32, bsl])
                    nc.vector.tensor_mul(u2[:], kiE[:], s4_sb[0:32, bsl])
                    nc.vector.tensor_mul(u3[:], krE[:], s4_sb[0:32, bsl])
                    nc.vector.tensor_mul(u4[:], kiE[:], c4_sb[0:32, bsl])
                    for g in (0, 64):
                        nc.vector.tensor_sub(
                            KRI2[g:g + 32, tb:tb + 1024], u1[:], u2[:])
                    for g in (32, 96):
                        nc.vector.tensor_add(
                            KRI2[g:g + 32, tb:tb + 1024], u3[:], u4[:])

            # -------------- phase 2: attention + wo, per qt-512 pair ----
            with tc.tile_pool(name="probs", bufs=3) as probsp, \
                 tc.tile_pool(name="attnp", bufs=2) as attnp, \
                 tc.tile_pool(name="normp", bufs=2) as normp, \
                 tc.tile_pool(name="outp", bufs=2) as outp, \
                 tc.tile_pool(name="sps", bufs=2, space="PSUM") as sps, \
                 tc.tile_pool(name="pvps", bufs=1, space="PSUM") as pvps, \
                 tc.tile_pool(name="ops", bufs=1, space="PSUM") as opsp:

                def emit_wo(attn01, attn23, qb):
                    for qs in range(4):
                        qq = qb + qs * 128
                        osb = outp.tile([128, D], BF16, name="osb")
                        for grp in range(2):
                            Og = [opsp.tile([128, 512], F32, name=f"Og{j}")
                                  for j in range(2)]
                            for part, at in ((0, attn01), (1, attn23)):
                                for j in range(2):
                                    do = grp * 2 + j
                                    nc.tensor.matmul(
                                        Og[j][:],
                                        at[:, qs * 128:(qs + 1) * 128],
                                        woT_r[:, part * D + do * 512:
                                              part * D + (do + 1) * 512],
                                        start=(part == 0), stop=(part == 1))
                            for j in range(2):
                                do = grp * 2 + j
                                (nc.vector.tensor_copy if j == 0 else
                                 nc.scalar.copy)(
                                    osb[:, do * 512:(do + 1) * 512],
                                    Og[j][:])
                        nc.sync.dma_start(o[qq:qq + 128, :], osb[:])

                wo_prev = None
                for b in range(2):
                    for jp in range(4):          # qt-512 blocks
                        qb = b * S + jp * 512
                        nkt = 4 * jp + 4
                        attn01 = attnp.tile([128, 512], BF16, name="at01")
                        attn23 = attnp.tile([128, 512], BF16, name="at23")
                        for pi, (QRI, attn) in enumerate(
                                ((QRI_A, attn01), (QRI_B, attn23))):
                            PVs = [pvps.tile([65, 512], F32, name=f"PV{hh}")
                                   for hh in range(2)]
                            pg_prev = None
                            for kt in range(nkt):
                                kc = b * S + kt * 128
                                r = kt - (nkt - 4)
                                half = r >= 2   # only right half live
                                Sg = sps.tile([128, 1024], F32, name="Sg")
                                pg = probsp.tile([128, 1024], BF16,
                                                 name="pg")
                                if not half:
                                    # hh0 -> cols 0:512, hh1 -> 512:1024
                                    for hh in range(2):
                                        nc.tensor.matmul(
                                            Sg[:, hh * 512:(hh + 1) * 512],
                                            KRI2[64 * hh:64 * hh + 64,
                                                 kc:kc + 128],
                                            QRI[64 * hh:64 * hh + 64,
                                                qb:qb + 512],
                                            start=True, stop=True,
                                            tile_position=(64 * hh, 0))
                                    nc.scalar.activation(
                                        pg[:], Sg[:], AF.Exp, scale=0.125)
                                else:
                                    # compact: hh0 -> 256:512, hh1 -> 512:768
                                    nc.tensor.matmul(
                                        Sg[:, 256:512],
                                        KRI2[0:64, kc:kc + 128],
                                        QRI[0:64, qb + 256:qb + 512],
                                        start=True, stop=True,
                                        tile_position=(0, 0))
                                    nc.tensor.matmul(
                                        Sg[:, 512:768],
                                        KRI2[64:128, kc:kc + 128],
                                        QRI[64:128, qb + 256:qb + 512],
                                        start=True, stop=True,
                                        tile_position=(64, 0))
                                    nc.scalar.activation(
                                        pg[:, 256:768], Sg[:, 256:768],
                                        AF.Exp, scale=0.125)
                                if r >= 0:
                                    if half:
                                        msl = slice(512 * r + 256,
                                                    512 * r + 512)
                                        sl0 = slice(256, 512)
                                        sl1 = slice(512, 768)
                                    else:
                                        msl = slice(512 * r, 512 * r + 256)
                                        sl0 = slice(0, 256)
                                        sl1 = slice(512, 768)
                                    nc.vector.tensor_mul(
                                        pg[:, sl0], pg[:, sl0],
                                        maskP_sb[:, msl])
                                    nc.vector.tensor_mul(
                                        pg[:, sl1], pg[:, sl1],
                                        maskP_sb[:, msl])
                                if pg_prev is not None:
                                    pkt, ppg, phalf = pg_prev
                                    vt = Vt3[:, b * 16 + pkt, :]
                                    st = (pkt == 0)
                                    for hh in range(2):
                                        if phalf:
                                            mv = ppg[:, 256 + 256 * hh:
                                                     512 + 256 * hh]
                                            dst = PVs[hh][:, 256:512]
                                        else:
                                            mv = ppg[:, hh * 512:
                                                     (hh + 1) * 512]
                                            dst = PVs[hh][:]
                                        nc.tensor.matmul(
                                            dst, vt, mv,
                                            start=st, stop=False)
                                pg_prev = (kt, pg, half)
                            pkt, ppg, phalf = pg_prev
                            vt = Vt3[:, b * 16 + pkt, :]
                            for hh in range(2):
                                if phalf:
                                    mv = ppg[:, 256 + 256 * hh:
                                             512 + 256 * hh]
                                    dst = PVs[hh][:, 256:512]
                                else:
                                    mv = ppg[:, hh * 512:(hh + 1) * 512]
                                    dst = PVs[hh][:]
                                nc.tensor.matmul(dst, vt, mv,
                                                 start=(pkt == 0), stop=True)
                            # normalization: 1/denom via ln -> exp(-x),
                            # both heads packed along the free dim
                            lnd = normp.tile([1, 1024], F32, name="lnd")
                            rec = normp.tile([1, 1024], F32, name="rec")
                            for hh in range(2):
                                nc.scalar.activation(
                                    lnd[:, hh * 512:(hh + 1) * 512],
                                    PVs[hh][64:65, :], AF.Ln)
                            nc.scalar.activation(
                                rec[:], lnd[:], AF.Exp, scale=-1.0)
                            for hh in range(2):
                                bcst = normp.tile([64, 512], F32,
                                                  name=f"bc{hh}")
                                nc.gpsimd.partition_broadcast(
                                    bcst[:], rec[:, hh * 512:(hh + 1) * 512])
                                nc.vector.tensor_mul(
                                    attn[64 * hh:64 * hh + 64, :],
                                    PVs[hh][0:64, :], bcst[:])
                            if pi == 0 and wo_prev is not None:
                                emit_wo(*wo_prev)
                                wo_prev = None
                        wo_prev = (attn01, attn23, qb)
                emit_wo(*wo_prev)

    nc.compile()
    return nc


def _prep_inputs(x, freqs_cos, freqs_sin, wq, wk, wv, wo):
    import ml_dtypes
    BF = ml_dtypes.bfloat16
    xf = np.ascontiguousarray(
        np.asarray(x, np.float32).reshape(T, D)).astype(BF)
    wq = np.asarray(wq, np.float32)
    wk = np.asarray(wk, np.float32)
    wv = np.asarray(wv, np.float32)
    wo = np.asarray(wo, np.float32)
    fc = np.asarray(freqs_cos, np.float32)
    fs = np.asarray(freqs_sin, np.float32)

    c4 = np.ascontiguousarray(np.tile(fc.T, (4, 1))).astype(BF)   # [128, S]
    s4 = np.ascontiguousarray(np.tile(fs.T, (4, 1))).astype(BF)
    kt = np.arange(128)[:, None]
    qt = np.arange(256)[None, :]
    mA = (kt <= qt).astype(np.float32)
    mB = (kt + 128 <= qt).astype(np.float32)
    one = np.ones((128, 256), np.float32)
    zero = np.zeros((128, 256), np.float32)
    maskP = np.concatenate([
        np.concatenate([mA, one], axis=1),
        np.concatenate([mB, one], axis=1),
        np.concatenate([zero, mA], axis=1),
        np.concatenate([zero, mB], axis=1)], axis=1).astype(BF)  # [128, 2048]
    ev = np.arange(0, 64, 2)
    od = np.arange(1, 64, 2)

    in_maps = []
    for c in range(NCORES):
        qreal = np.concatenate([(4 * c + h) * 64 + ev for h in range(4)])
        qimag = np.concatenate([(4 * c + h) * 64 + od for h in range(4)])
        Wc = np.concatenate([wq[qreal], wq[qimag], wk[c * 64 + ev],
                             wk[c * 64 + od], wv[c * 64:(c + 1) * 64]], axis=0)
        in_maps.append({
            "x": xf,
            "wqkvT": np.ascontiguousarray(Wc.T).astype(BF),
            "woT": np.ascontiguousarray(
                wo[:, c * 256:(c + 1) * 256].T).astype(BF),
            "c4": c4, "s4": s4, "maskP": maskP,
        })
    return in_maps


def _run(in_maps, trace=False, **kw):
    from concourse import bass_utils
    if "nc" not in _cache:
        _cache["nc"] = _build()
    return bass_utils.run_bass_kernel_spmd(
        _cache["nc"], in_maps, core_ids=list(range(NCORES)), trace=trace, **kw)


def kernel(x, freqs_cos, freqs_sin, wq, wk, wv, wo):
    in_maps = _prep_inputs(x, freqs_cos, freqs_sin, wq, wk, wv, wo)
    for _ in range(3):
        res = _run(in_maps)
        out = res.results[0]["o"].astype(np.float64)
        for c in range(1, NCORES):
            out += res.results[c]["o"].astype(np.float64)
        if np.isfinite(out).all():
            break
    return out.astype(np.float32).reshape(B, S, D)


# revision 26
# speedup vs baseline: 1.2071x; 1.2071x over previous
"""Trainium2 Bass kernel for nn_Attention_14542759264705.

Dense transformer attention: QKV proj + interleaved RoPE + GQA causal
attention (32 q heads / 8 kv heads, hd=64) + output proj, fp32 in/out.

Sharding: tensor-parallel over kv-head groups across 8 cores. Core c owns
q heads 4c..4c+3 and kv head c; each core computes a partial output and
the host sums the 8 partials.

v2: bf16 datapath (fp32 PSUM accumulation everywhere):
  - x converted to bf16 on host; xT loaded straight into SBUF via
    DMA-transpose (removes 512 PE transposes + 128 ACT evacuations).
  - weights/Q/K/V/probs/attn/masks/output all bf16; rope DVE ops run in
    bf16 2x mode; mask muls 2x.
  - proj: 1024-token chunks, 6 psum accumulators, fast evac to SBUF
    (3 DVE + 3 ACT copies) then rope reads SBUF.
  - scores: per kt one [128,1024] 2-bank psum tile (two heads), single
    paired exp; half-width diagonal tiles compacted to the middle 512
    cols so one exp covers both heads.
  - softmax denominators: reciprocal via ACT ln->exp(-x) (single
    natural_log_exp table set) instead of 3.3us DVE reciprocals.
  - wo: 2-do-group accumulation, evac alternating DVE/ACT, bf16 output
    summed on host in fp32.
"""
import numpy as np

B, S, D = 2, 2048, 2048
T = B * S
NH, NKV, HD = 32, 8, 64
NCORES = 8

_cache = {}


def _build():
    import concourse.bacc as bacc
    import concourse.mybir as mybir
    import concourse.tile as tile
    from concourse.masks import make_identity

    F32 = mybir.dt.float32
    BF16 = mybir.dt.bfloat16
    AF = mybir.ActivationFunctionType

    nc = bacc.Bacc("TRN2", target_bir_lowering=False, debug=False,
                   num_devices=NCORES)
    # Steer the act-table pass to the set holding BOTH exp and ln, so the
    # per-pass ln->exp reciprocals don't thrash ACT_TABLE_LOADs: hide Exp
    # from every other set (the chosen set genuinely contains both, so the
    # emitted program is unchanged in semantics, just uses one table).
    from concourse.hw_specs import get_activation_tables
    tabs = get_activation_tables(nc.m.arch)
    for name, fns in tabs.items():
        if name != "natural_log_exp_and_others":
            fns.discard(AF.Exp)
    x = nc.dram_tensor("x", [T, D], BF16, kind="ExternalInput").ap()
    wqkvT = nc.dram_tensor("wqkvT", [D, 384], BF16, kind="ExternalInput").ap()
    woT = nc.dram_tensor("woT", [256, D], BF16, kind="ExternalInput").ap()
    c4 = nc.dram_tensor("c4", [128, S], BF16, kind="ExternalInput").ap()
    s4 = nc.dram_tensor("s4", [128, S], BF16, kind="ExternalInput").ap()
    maskP = nc.dram_tensor("maskP", [128, 4 * 512], BF16,
                           kind="ExternalInput").ap()
    o = nc.dram_tensor("o", [T, D], BF16, kind="ExternalOutput").ap()

    with tile.TileContext(nc) as tc:
        with tc.tile_pool(name="resident", bufs=1) as res:
            ident = res.tile([128, 128], BF16)
            make_identity(nc, ident[:])
            c4_sb = res.tile([128, S], BF16)
            s4_sb = res.tile([128, S], BF16)
            maskP_sb = res.tile([128, 4 * 512], BF16)
            nc.sync.dma_start(c4_sb[:], c4[:])
            nc.sync.dma_start(s4_sb[:], s4[:])
            nc.sync.dma_start(maskP_sb[:], maskP[:])

            QRI_A = res.tile([128, T], BF16)   # [h0r h0i h1r h1i] x tokens
            QRI_B = res.tile([128, T], BF16)   # [h2r h2i h3r h3i]
            KRI2 = res.tile([128, T], BF16)    # [Kr Ki Kr Ki]
            Vt_sb = res.tile([128, 32 * 65], BF16)  # kt-tile k at cols k*65
            Vt3 = Vt_sb.rearrange("p (k c) -> p k c", c=65)
            wqkv_r = res.tile([128, 16 * 384], BF16)
            woT_r = res.tile([128, 2 * D], BF16)
            ones32 = res.tile([128, 32], BF16)
            nc.gpsimd.memset(ones32[:], 1.0)
            nc.vector.tensor_copy(Vt3[:, :, 64], ones32[:])

            for d in range(16):
                nc.sync.dma_start(wqkv_r[:, d * 384:(d + 1) * 384],
                                  wqkvT[d * 128:(d + 1) * 128, :])
            for t in range(2):
                nc.sync.dma_start(woT_r[:, t * D:(t + 1) * D],
                                  woT[t * 128:(t + 1) * 128, :])

            # ---------------- phase 1: xT dma, proj, rope --------------
            with tc.tile_pool(name="xTp", bufs=3) as xTp, \
                 tc.tile_pool(name="evacp", bufs=2) as evp, \
                 tc.tile_pool(name="ropet", bufs=2) as rp, \
                 tc.tile_pool(name="projps", bufs=1, space="PSUM") as projp, \
                 tc.tile_pool(name="vtps", bufs=1, space="PSUM") as vtp_pool:

                for J in range(4):            # 1024-token chunks
                    tb = J * 1024
                    bc = (J % 2) * 1024       # within-batch col base
                    xts = []
                    for d in range(16):
                        xt = xTp.tile([128, 1024], BF16, name=f"xt{d % 3}")
                        nc.sync.dma_start_transpose(
                            xt[:], x[tb:tb + 1024, d * 128:(d + 1) * 128])
                        xts.append(xt)
                    ps = {}
                    for ch, nm in ((0, "QR"), (1, "QI"), (2, "KV")):
                        for h in range(2):
                            ps[(ch, h)] = projp.tile(
                                [128, 512], F32, name=f"{nm}{h}")
                    for d in range(16):
                        for ch in range(3):
                            w = wqkv_r[:, d * 384 + ch * 128:
                                       d * 384 + (ch + 1) * 128]
                            for h in range(2):
                                nc.tensor.matmul(
                                    ps[(ch, h)][:],
                                    w, xts[d][:, h * 512:(h + 1) * 512],
                                    start=(d == 0), stop=(d == 15))
    # fast psum evac (bf16), split DVE/ACT; K and V land on
                    # partition-0-based tiles (V transpose needs base 0)
                    qrE = evp.tile([128, 1024], BF16, name="qrE")
                    qiE = evp.tile([128, 1024], BF16, name="qiE")
                    krE = evp.tile([32, 1024], BF16, name="krE")
                    kiE = evp.tile([32, 1024], BF16, name="kiE")
                    vE = evp.tile([64, 1024], BF16, name="vE")
                    for h in range(2):
                        sl = slice(h * 512, (h + 1) * 512)
                        nc.vector.tensor_copy(qrE[:, sl], ps[(0, h)][:])
                        nc.scalar.copy(qiE[:, sl], ps[(1, h)][:])
                        nc.vector.tensor_copy(krE[:, sl], ps[(2, h)][0:32, :])
                        nc.vector.tensor_copy(kiE[:, sl],
                                              ps[(2, h)][32:64, :])
                        nc.scalar.copy(vE[:, sl], ps[(2, h)][64:128, :])
                    # V transposes
                    for i in range(8):
                        vtp = vtp_pool.tile([128, 64], BF16, name="vtp")
                        nc.tensor.transpose(
                            vtp[:], vE[:, i * 128:(i + 1) * 128],
                            ident[0:64, 0:64])
                        nc.scalar.copy(Vt3[:, J * 8 + i, 0:64], vtp[:])
                    # rope (bf16 DVE 2x mode)
                    cs = c4_sb[:, bc:bc + 1024]
                    sn = s4_sb[:, bc:bc + 1024]
                    t1 = rp.tile([128, 1024], BF16, name="t1")
                    t2 = rp.tile([128, 1024], BF16, name="t2")
                    t3 = rp.tile([128, 1024], BF16, name="t3")
                    t4 = rp.tile([128, 1024], BF16, name="t4")
                    qtr = rp.tile([128, 1024], BF16, name="qtr")
                    qti = rp.tile([128, 1024], BF16, name="qti")
                    nc.vector.tensor_mul(t1[:], qrE[:], cs)
                    nc.vector.tensor_mul(t2[:], qiE[:], sn)
                    nc.vector.tensor_mul(t3[:], qrE[:], sn)
                    nc.vector.tensor_mul(t4[:], qiE[:], cs)
                    nc.vector.tensor_sub(qtr[:], t1[:], t2[:])
                    nc.vector.tensor_add(qti[:], t3[:], t4[:])
                    for hh in range(4):
                        dst = QRI_A if hh < 2 else QRI_B
                        base = (hh % 2) * 64
                        nc.vector.tensor_copy(
                            dst[base:base + 32, tb:tb + 1024],
                            qtr[32 * hh:32 * hh + 32, :])
                        nc.vector.tensor_copy(
                            dst[base + 32:base + 64, tb:tb + 1024],
                            qti[32 * hh:32 * hh + 32, :])
                    # K rope: kE rows = [Kr; Ki]; all u tiles base-0 so
                    # the add/sub inputs share a base partition
                    u1 = rp.tile([32, 1024], BF16, name="u1")
                    u2 = rp.tile([32, 1024], BF16, name="u2")
                    u3 = rp.tile([32, 1024], BF16, name="u3")
                    u4 = rp.tile([32, 1024], BF16, name="u4")
                    bsl = slice(bc, bc + 1024)
                    nc.vector.tensor_mul(u1[:], krE[:], c4_sb[0:32, bsl])
                    nc.vector.tensor_mul(u2[:], kiE[:], s4_sb[0:32, bsl])
                    nc.vector.tensor_mul(u3[:], krE[:], s4_sb[0:32, bsl])
                    nc.vector.tensor_mul(u4[:], kiE[:], c4_sb[0:32, bsl])
                    for g in (0, 64):
                        nc.vector.tensor_sub(
                            KRI2[g:g + 32, tb:tb + 1024], u1[:], u2[:])
                    for g in (32, 96):
                        nc.vector.tensor_add(
                            KRI2[g:g + 32, tb:tb + 1024], u3[:], u4[:])

            # -------------- phase 2: attention + wo, per qt-512 pair ----
            with tc.tile_pool(name="probs", bufs=3) as probsp, \
                 tc.tile_pool(name="attnp", bufs=2) as attnp, \
                 tc.tile_pool(name="normp", bufs=2) as normp, \
                 tc.tile_pool(name="outp", bufs=2) as outp, \
                 tc.tile_pool(name="sps", bufs=2, space="PSUM") as sps, \
                 tc.tile_pool(name="pvps", bufs=1, space="PSUM") as pvps, \
                 tc.tile_pool(name="ops", bufs=1, space="PSUM") as opsp:

                def emit_wo(attn01, attn23, qb):
                    for qs in range(4):
                        qq = qb + qs * 128
                        osb = outp.tile([128, D], BF16, name="osb")
                        for grp in range(2):
                            Og = [opsp.tile([128, 512], F32, name=f"Og{j}")
                                  for j in range(2)]
                            for part, at in ((0, attn01), (1, attn23)):
                                for j in range(2):
                                    do = grp * 2 + j
                                    nc.tensor.matmul(
                                        Og[j][:],
                                        at[:, qs * 128:(qs + 1) * 128],
                                        woT_r[:, part * D + do * 512:
                                              part * D + (do + 1) * 512],
                                        start=(part == 0), stop=(part == 1))
                            for j in range(2):
                                do = grp * 2 + j
                                (nc.vector.tensor_copy if j == 0 else
                                 nc.scalar.copy)(
                                    osb[:, do * 512:(do + 1) * 512],
                                    Og[j][:])
                        nc.sync.dma_start(o[qq:qq + 128, :], osb[:])

                wo_prev = None
                for b in range(2):
                    for jp in range(4):          # qt-512 blocks
                        qb = b * S + jp * 512
                        nkt = 4 * jp + 4
                        attn01 = attnp.tile([128, 512], BF16, name="at01")
                        attn23 = attnp.tile([128, 512], BF16, name="at23")
                        for pi, (QRI, attn) in enumerate(
                                ((QRI_A, attn01), (QRI_B, attn23))):
                            PVs = [pvps.tile([65, 512], F32, name=f"PV{hh}")
                                   for hh in range(2)]
                            pg_prev = None
                            for kt in range(nkt):
                                kc = b * S + kt * 128
                                r = kt - (nkt - 4)
                                half = r >= 2   # only right half live
                                Sg = sps.tile([128, 1024], F32, name="Sg")
                                pg = probsp.tile([128, 1024], BF16,
                                                 name="pg")
                                if not half:
                                    # hh0 -> cols 0:512, hh1 -> 512:1024
                                    for hh in range(2):
                                        nc.tensor.matmul(
                                            Sg[:, hh * 512:(hh + 1) * 512],
                                            KRI2[64 * hh:64 * hh + 64,
                                                 kc:kc + 128],
                                            QRI[64 * hh:64 * hh + 64,
                                                qb:qb + 512],
                                            start=True, stop=True,
                                            tile_position=(64 * hh, 0))
                                    nc.scalar.activation(
                                        pg[:], Sg[:], AF.Exp, scale=0.125)
                                else:
                                    # compact: hh0 -> 256:512, hh1 -> 512:768
                                    nc.tensor.matmul(
                                        Sg[:, 256:512],
                                        KRI2[0:64, kc:kc + 128],
                                        QRI[0:64, qb + 256:qb + 512],
                                        start=True, stop=True,
                                        tile_position=(0, 0))
                                    nc.tensor.matmul(
                                        Sg[:, 512:768],
                                        KRI2[64:128, kc:kc + 128],
                                        QRI[64:128, qb + 256:qb + 512],
                                        start=True, stop=True,
                                        tile_position=(64, 0))
                                    nc.scalar.activation(
                                        pg[:, 256:768], Sg[:, 256:768],
                                        AF.Exp, scale=0.125)
                                if r >= 0:
                                    if half:
                                        msl = slice(512 * r + 256,
                                                    512 * r + 512)
                                        sl0 = slice(256, 512)
                                        sl1 = slice(512, 768)
                                    else:
                                        msl = slice(512 * r, 512 * r + 256)
                                        sl0 = slice(0, 256)
                                        sl1 = slice(512, 768)
                                    nc.vector.tensor_mul(
                                        pg[:, sl0], pg[:, sl0],
                                        maskP_sb[:, msl])
                                    nc.vector.tensor_mul(
                                        pg[:, sl1], pg[:, sl1],
                                        maskP_sb[:, msl])
                                if pg_prev is not None:
                                    pkt, ppg, phalf = pg_prev
                                    vt = Vt3[:, b * 16 + pkt, :]
                                    st = (pkt == 0)
                                    for hh in range(2):
                                        if phalf:
                                            mv = ppg[:, 256 + 256 * hh:
                                                     512 + 256 * hh]
                                            dst = PVs[hh][:, 256:512]
                                        else:
                                            mv = ppg[:, hh * 512:
                                                     (hh + 1) * 512]
                                            dst = PVs[hh][:]
                                        nc.tensor.matmul(
                                            dst, vt, mv,
                                            start=st, stop=False)
                                pg_prev = (kt, pg, half)
                            pkt, ppg, phalf = pg_prev
                            vt = Vt3[:, b * 16 + pkt, :]
                            for hh in range(2):
                                if phalf:
                                    mv = ppg[:, 256 + 256 * hh:
                                             512 + 256 * hh]
                                    dst = PVs[hh][:, 256:512]
                                else:
                                    mv = ppg[:, hh * 512:(hh + 1) * 512]
                                    dst = PVs[hh][:]
                                nc.tensor.matmul(dst, vt, mv,
                                                 start=(pkt == 0), stop=True)
                            # normalization: 1/denom via ln -> exp(-x),
                            # both heads packed along the free dim
                            lnd = normp.tile([1, 1024], F32, name="lnd")
                            rec = normp.tile([1, 1024], F32, name="rec")
                            for hh in range(2):
                                nc.scalar.activation(
                                    lnd[:, hh * 512:(hh + 1) * 512],
                                    PVs[hh][64:65, :], AF.Ln)
                            nc.scalar.activation(
                                rec[:], lnd[:], AF.Exp, scale=-1.0)
                            for hh in range(2):
                                bcst = normp.tile([64, 512], F32,
                                                  name=f"bc{hh}")
                                nc.gpsimd.partition_broadcast(
                                    bcst[:], rec[:, hh * 512:(hh + 1) * 512])
                                nc.vector.tensor_mul(
                                    attn[64 * hh:64 * hh + 64, :],
                                    PVs[hh][0:64, :], bcst[:])
                            if pi == 0 and wo_prev is not None:
                                emit_wo(*wo_prev)
                                wo_prev = None
                        wo_prev = (attn01, attn23, qb)
                emit_wo(*wo_prev)

    nc.compile()
    return nc


def _prep_inputs(x, freqs_cos, freqs_sin, wq, wk, wv, wo):
    import ml_dtypes
    BF = ml_dtypes.bfloat16
    xf = np.ascontiguousarray(
        np.asarray(x, np.float32).reshape(T, D)).astype(BF)
    wq = np.asarray(wq, np.float32)
    wk = np.asarray(wk, np.float32)
    wv = np.asarray(wv, np.float32)
    wo = np.asarray(wo, np.float32)
    fc = np.asarray(freqs_cos, np.float32)
    fs = np.asarray(freqs_sin, np.float32)

    c4 = np.ascontiguousarray(np.tile(fc.T, (4, 1))).astype(BF)   # [128, S]
    s4 = np.ascontiguousarray(np.tile(fs.T, (4, 1))).astype(BF)
    kt = np.arange(128)[:, None]
    qt = np.arange(256)[None, :]
    mA = (kt <= qt).astype(np.float32)
    mB = (kt + 128 <= qt).astype(np.float32)
    one = np.ones((128, 256), np.float32)
    zero = np.zeros((128, 256), np.float32)
    maskP = np.concatenate([
        np.concatenate([mA, one], axis=1),
        np.concatenate([mB, one], axis=1),
        np.concatenate([zero, mA], axis=1),
        np.concatenate([zero, mB], axis=1)], axis=1).astype(BF)  # [128, 2048]
    ev = np.arange(0, 64, 2)
    od = np.arange(1, 64, 2)

    in_maps = []
    for c in range(NCORES):
        qreal = np.concatenate([(4 * c + h) * 64 + ev for h in range(4)])
        qimag = np.concatenate([(4 * c + h) * 64 + od for h in range(4)])
        Wc = np.concatenate([wq[qreal], wq[qimag], wk[c * 64 + ev],
                             wk[c * 64 + od], wv[c * 64:(c + 1) * 64]], axis=0)
        in_maps.append({
            "x": xf,
            "wqkvT": np.ascontiguousarray(Wc.T).astype(BF),
            "woT": np.ascontiguousarray(
                wo[:, c * 256:(c + 1) * 256].T).astype(BF),
            "c4": c4, "s4": s4, "maskP": maskP,
        })
    return in_maps


def _run(in_maps, trace=False, **kw):
    from concourse import bass_utils
    if "nc" not in _cache:
        _cache["nc"] = _build()
    return bass_utils.run_bass_kernel_spmd(
        _cache["nc"], in_maps, core_ids=list(range(NCORES)), trace=trace, **kw)


def kernel(x, freqs_cos, freqs_sin, wq, wk, wv, wo):
    in_maps = _prep_inputs(x, freqs_cos, freqs_sin, wq, wk, wv, wo)
    for _ in range(3):
        res = _run(in_maps)
        out = res.results[0]["o"].astype(np.float64)
        for c in range(1, NCORES):
            out += res.results[c]["o"].astype(np.float64)
        if np.isfinite(out).all():
            break
    return out.astype(np.float32).reshape(B, S, D)
